# revision 12
# baseline (speedup 1.0000x reference)
"""Trainium2 Bass kernel for nn_CrossAttention_15006615733765 (raw Bass, no Tile).

Mathematical structure: the reference broadcasts a per-batch context vector
(B, CTX_DIM) to every spatial position before projecting to K/V.  All keys
within a batch are therefore identical, softmax over the key axis is exactly
uniform, and the attention output equals V itself.  The module collapses to

    out[b, c, h, w] = ((context[b] @ Wv) @ Wo + bo)[c]

independent of x, Wq and Wk (exact in infinite precision; measured rel err
vs the fp32 reference is ~4e-7).  The kernel computes the two small matmuls
on the tensor engine and materializes the broadcast output shard per core,
sharding the 512 output channels across 8 cores.

Same math and layouts as kernel.py v2, but with hand-placed semaphores to
avoid the Tile framework's startup barrier phase (~7 us) and kernel-tail
EVSEM butterfly (~8.5 us).

Engine plan:
  Sync   : ctx + 6 Wv chunk loads (HWDGE), final broadcast store, end-wait
  Scalar : wo / sel / id / bo const loads (second HWDGE queue)
  Tensor : stage1 matmuls -> transposes -> stage2 -> selector broadcast
  Vector : PSUM -> SBUF copies between PE stages
  GpSimd : unused (block exits with no_gpsimd_drain)
"""

import numpy as np

import concourse.bacc as bacc
import concourse.mybir as mybir
from concourse.bass_utils import run_bass_kernel_spmd

B, DIM, CTX_DIM = 4, 512, 768
H = W = 48
NPOS = H * W
NCORES = 8
CPC = DIM // NCORES
P = 128
KC = CTX_DIM // P
KD = DIM // P
NREP = NPOS // P
ROW = B * CPC
F32 = mybir.dt.float32

_CACHE: dict = {}


def _build_nc():
    nc = bacc.Bacc("TRN2", target_bir_lowering=False, debug=False, num_devices=NCORES)

    ctxc = nc.dram_tensor("ctxc", [P, KC, B], F32, kind="ExternalInput")
    wvc = nc.dram_tensor("wvc", [P, KC, DIM], F32, kind="ExternalInput")
    woc = nc.dram_tensor("woc", [P, KD, CPC], F32, kind="ExternalInput")
    selc = nc.dram_tensor("selc", [B + 1, B, P], F32, kind="ExternalInput")
    idc = nc.dram_tensor("idc", [B, B], F32, kind="ExternalInput")
    boc = nc.dram_tensor("boc", [1, CPC], F32, kind="ExternalInput")
    outd = nc.dram_tensor("outd", [NPOS, ROW], F32, kind="ExternalOutput")

    ctx_sb = nc.alloc_sbuf_tensor("ctx_sb", [P, KC, B], F32).ap()
    wv_sb = nc.alloc_sbuf_tensor("wv_sb", [P, KC, DIM], F32).ap()
    wo_sb = nc.alloc_sbuf_tensor("wo_sb", [P, KD, CPC], F32).ap()
    sel_sb = nc.alloc_sbuf_tensor("sel_sb", [B + 1, B, P], F32).ap()
    id_sb = nc.alloc_sbuf_tensor("id_sb", [B, B], F32).ap()
    o5_sb = nc.alloc_sbuf_tensor("o5_sb", [B + 1, CPC], F32).ap()
    t_sb = nc.alloc_sbuf_tensor("t_sb", [B, DIM], F32).ap()
    tT_sb = nc.alloc_sbuf_tensor("tT_sb", [P, KD, B], F32).ap()
    # Output row duplicated twice per partition -> 2 KiB DMA descriptors.
    rep_sb = nc.alloc_sbuf_tensor("rep_sb", [P, 2, ROW], F32).ap()

    pt = nc.alloc_psum_tensor("pt", [B, DIM], F32).ap()
    ptT = nc.alloc_psum_tensor("ptT", [P, KD, B], F32).ap()
    po = nc.alloc_psum_tensor("po", [B, CPC], F32).ap()
    prep = nc.alloc_psum_tensor("prep", [P, B, CPC], F32).ap()
    pwarm = nc.alloc_psum_tensor("pwarm", [B, DIM], F32).ap()

    from contextlib import ExitStack

    with ExitStack() as stack:
        s_ctx = stack.enter_context(nc.semaphore("s_ctx"))
        s_wv = [stack.enter_context(nc.semaphore(f"s_wv{k}")) for k in range(KC)]
        s_const = stack.enter_context(nc.semaphore("s_const"))
        s_mm = stack.enter_context(nc.semaphore("s_mm"))
        s_tcp = stack.enter_context(nc.semaphore("s_tcp"))
        s_tTcp = stack.enter_context(nc.semaphore("s_tTcp"))
        s_o5 = stack.enter_context(nc.semaphore("s_o5"))
        s_rep = stack.enter_context(nc.semaphore("s_rep"))
        s_out = stack.enter_context(nc.semaphore("s_out"))

        with nc.Block(no_gpsimd_drain=True) as block:

            @block.sync
            def _(sync):
                sync.dma_start(
                    out=wv_sb[:, 0, :], in_=wvc[:, 0, :]
                ).then_inc(s_wv[0], 16)
                sync.dma_start(out=ctx_sb[:], in_=ctxc[:]).then_inc(s_ctx, 16)
                for k in range(1, KC):
                    sync.dma_start(
                        out=wv_sb[:, k, :], in_=wvc[:, k, :]
                    ).then_inc(s_wv[k], 16)
                sync.wait_ge(s_rep, 1)
                # pos = r*256 + p*2 + d: each partition contributes 2048-byte
                # contiguous chunks (two consecutive 256-float rows).  Split
                # across the sync and scalar HWDGE queues by partition half.
                out_view = outd.rearrange("(r p d) n -> p r (d n)", p=P, d=2)
                src_view = (
                    rep_sb.rearrange("p d n -> p (d n)")[:, None, :]
                    .broadcast_to((P, NPOS // (2 * P), 2 * ROW))
                )
                sync.dma_start(
                    out=out_view[:P // 2], in_=src_view[:P // 2]
                ).then_inc(s_out, 16)
                sync.wait_ge(s_out, 32)

            @block.scalar
            def _(scalar):
                scalar.dma_start(out=wo_sb[:], in_=woc[:]).then_inc(s_const, 16)
                scalar.dma_start(out=sel_sb[:], in_=selc[:]).then_inc(s_const, 16)
                scalar.dma_start(out=id_sb[:], in_=idc[:]).then_inc(s_const, 16)
                scalar.dma_start(
                    out=o5_sb[B:B + 1, :], in_=boc[:]
                ).then_inc(s_const, 16)
                scalar.wait_ge(s_rep, 1)
                out_view = outd.rearrange("(r p d) n -> p r (d n)", p=P, d=2)
                src_view = (
                    rep_sb.rearrange("p d n -> p (d n)")[:, None, :]
                    .broadcast_to((P, NPOS // (2 * P), 2 * ROW))
                )
                scalar.dma_start(
                    out=out_view[P // 2:], in_=src_view[P // 2:]
                ).then_inc(s_out, 16)

            @block.tensor
            def _(tensor):
                # HAM warmup: ungated dummy matmuls (garbage SBUF data,
                # scratch PSUM) keep the PE busy from boot until the first
                # Wv chunk lands, ramping the PE clock from 1.2 to 2.4 GHz.
                for _w in range(2):
                    nc.tensor.matmul(
                        pwarm[:],
                        ctx_sb[:, 0, :],
                        wv_sb[:, KC - 1, :],
                        start=True,
                        stop=True,
                    )
                tensor.wait_ge(s_ctx, 16)
                for k in range(KC):
                    tensor.wait_ge(s_wv[k], 16)
                    ins = nc.tensor.matmul(
                        pt[:],
                        ctx_sb[:, k, :],
                        wv_sb[:, k, :],
                        start=(k == 0),
                        stop=(k == KC - 1),
                    )
                ins.then_inc(s_mm, 1)

                tensor.wait_ge(s_tcp, 1)
                tensor.wait_ge(s_const, 64)
                for m in range(KD):
                    ins = nc.tensor.transpose(
                        ptT[:, m, :], t_sb[:, m * P:(m + 1) * P], id_sb[:]
                    )
                ins.then_inc(s_mm, 1)

                tensor.wait_ge(s_tTcp, 1)
                for m in range(KD):
                    ins = nc.tensor.matmul(
                        po[:],
                        tT_sb[:, m, :],
                        wo_sb[:, m, :],
                        start=(m == 0),
                        stop=(m == KD - 1),
                    )
                ins.then_inc(s_mm, 1)

                tensor.wait_ge(s_o5, 1)
                for b in range(B):
                    ins = nc.tensor.matmul(
                        prep[:, b, :],
                        sel_sb[:, b, :],
                        o5_sb[:, :],
                        start=True,
                        stop=True,
                    )
                ins.then_inc(s_mm, 1)

            @block.vector
            def _(vector):
                vector.wait_ge(s_mm, 1)
                nc.vector.tensor_copy(t_sb[:], pt[:]).then_inc(s_tcp, 1)
                vector.wait_ge(s_mm, 2)
                nc.vector.tensor_copy(tT_sb[:], ptT[:]).then_inc(s_tTcp, 1)
                vector.wait_ge(s_mm, 3)
                nc.vector.tensor_copy(o5_sb[:B, :], po[:]).then_inc(s_o5, 1)
                vector.wait_ge(s_mm, 4)
                flat = prep[:].rearrange("p b c -> p (b c)")
                nc.vector.tensor_copy(rep_sb[:, 0, :], flat)
                nc.vector.tensor_copy(rep_sb[:, 1, :], flat).then_inc(s_rep, 1)

    nc.compile()
    return nc


def _get_nc():
    if "nc" not in _CACHE:
        _CACHE["nc"] = _build_nc()
    return _CACHE["nc"]


def _prepare_in_maps(context, Wv, Wo, bo):
    context = np.ascontiguousarray(context, dtype=np.float32)
    Wv = np.ascontiguousarray(Wv, dtype=np.float32)
    Wo = np.ascontiguousarray(Wo, dtype=np.float32)
    bo = np.ascontiguousarray(bo, dtype=np.float32)

    ctxc = np.ascontiguousarray(context.T.reshape(KC, P, B).transpose(1, 0, 2))
    wvc = np.ascontiguousarray(Wv.reshape(KC, P, DIM).transpose(1, 0, 2))
    wo_chunk = Wo.reshape(KD, P, DIM).transpose(1, 0, 2)

    selc = np.zeros((B + 1, B, P), dtype=np.float32)
    for b in range(B):
        selc[b, b, :] = 1.0
        selc[B, b, :] = 1.0
    idc = np.eye(B, dtype=np.float32)

    in_maps = []
    for i in range(NCORES):
        woc = np.ascontiguousarray(wo_chunk[:, :, i * CPC:(i + 1) * CPC])
        boc = np.ascontiguousarray(bo[i * CPC:(i + 1) * CPC]).reshape(1, CPC)
        in_maps.append(
            {
                "ctxc": ctxc,
                "wvc": wvc,
                "woc": woc,
                "selc": selc,
                "idc": idc,
                "boc": boc,
            }
        )
    return in_maps


def _unshard(results):
    shards = np.stack([r["outd"] for r in results], axis=0)
    shards = shards.reshape(NCORES, NPOS, B, CPC)
    out = shards.transpose(2, 0, 3, 1).reshape(B, DIM, H, W)
    return np.ascontiguousarray(out)


def kernel(x, context, Wq, Wk, Wv, Wo, bo):
    del x, Wq, Wk
    nc = _get_nc()
    in_maps = _prepare_in_maps(context, Wv, Wo, bo)
    results = run_bass_kernel_spmd(nc, in_maps, list(range(NCORES))).results
    return _unshard(results)


# revision 14
# speedup vs baseline: 1.1720x; 1.1720x over previous
"""Trainium2 Bass kernel for nn_CrossAttention_15006615733765 (raw Bass, no Tile).

Mathematical structure: the reference broadcasts a per-batch context vector
(B, CTX_DIM) to every spatial position before projecting to K/V.  All keys
within a batch are therefore identical, softmax over the key axis is exactly
uniform, and the attention output equals V itself.  The module collapses to

    out[b, c, h, w] = ((context[b] @ Wv) @ Wo + bo)[c]

independent of x, Wq and Wk (exact in infinite precision; measured rel err
vs the fp32 reference is ~4e-7).  The kernel computes the two small matmuls
on the tensor engine and materializes the broadcast output shard per core,
sharding the 512 output channels across 8 cores.

Same math and layouts as kernel.py v2, but with hand-placed semaphores to
avoid the Tile framework's startup barrier phase (~7 us) and kernel-tail
EVSEM butterfly (~8.5 us).

Engine plan:
  Sync   : ctx + 6 Wv chunk loads (HWDGE), final broadcast store, end-wait
  Scalar : wo / sel / id / bo const loads (second HWDGE queue)
  Tensor : stage1 matmuls -> transposes -> stage2 -> selector broadcast
  Vector : PSUM -> SBUF copies between PE stages
  GpSimd : unused (block exits with no_gpsimd_drain)
"""

import numpy as np

import concourse.bacc as bacc
import concourse.mybir as mybir
from concourse.bass_utils import run_bass_kernel_spmd

B, DIM, CTX_DIM = 4, 512, 768
H = W = 48
NPOS = H * W
NCORES = 8
CPC = DIM // NCORES
P = 128
KC = CTX_DIM // P
KD = DIM // P
NREP = NPOS // P
ROW = B * CPC
F32 = mybir.dt.float32

_CACHE: dict = {}


def _build_nc():
    nc = bacc.Bacc("TRN2", target_bir_lowering=False, debug=False, num_devices=NCORES)

    ctxc = nc.dram_tensor("ctxc", [P, KC, B], F32, kind="ExternalInput")
    wvc = nc.dram_tensor("wvc", [P, KC, DIM], F32, kind="ExternalInput")
    woc = nc.dram_tensor("woc", [P, KD, CPC], F32, kind="ExternalInput")
    selc = nc.dram_tensor("selc", [B + 1, B, P], F32, kind="ExternalInput")
    idc = nc.dram_tensor("idc", [B, B], F32, kind="ExternalInput")
    boc = nc.dram_tensor("boc", [1, CPC], F32, kind="ExternalInput")
    outd = nc.dram_tensor("outd", [NPOS, ROW], F32, kind="ExternalOutput")

    ctx_sb = nc.alloc_sbuf_tensor("ctx_sb", [P, KC, B], F32).ap()
    wv_sb = nc.alloc_sbuf_tensor("wv_sb", [P, KC, DIM], F32).ap()
    wo_sb = nc.alloc_sbuf_tensor("wo_sb", [P, KD, CPC], F32).ap()
    sel_sb = nc.alloc_sbuf_tensor("sel_sb", [B + 1, B, P], F32).ap()
    id_sb = nc.alloc_sbuf_tensor("id_sb", [B, B], F32).ap()
    o5_sb = nc.alloc_sbuf_tensor("o5_sb", [B + 1, CPC], F32).ap()
    t_sb = nc.alloc_sbuf_tensor("t_sb", [B, DIM], F32).ap()
    tT_sb = nc.alloc_sbuf_tensor("tT_sb", [P, KD, B], F32).ap()
    # Output row duplicated twice per partition -> 2 KiB DMA descriptors.
    rep_sb = nc.alloc_sbuf_tensor("rep_sb", [P, 2, ROW], F32).ap()

    pt = nc.alloc_psum_tensor("pt", [B, DIM], F32).ap()
    ptT = nc.alloc_psum_tensor("ptT", [P, KD, B], F32).ap()
    po = nc.alloc_psum_tensor("po", [B, CPC], F32).ap()
    prep = nc.alloc_psum_tensor("prep", [P, B, CPC], F32).ap()
    pwarm = nc.alloc_psum_tensor("pwarm", [B, DIM], F32).ap()

    from contextlib import ExitStack

    with ExitStack() as stack:
        s_ctx = stack.enter_context(nc.semaphore("s_ctx"))
        s_wv = [stack.enter_context(nc.semaphore(f"s_wv{k}")) for k in range(KC)]
        s_const = stack.enter_context(nc.semaphore("s_const"))
        s_mm = stack.enter_context(nc.semaphore("s_mm"))
        s_tcp = stack.enter_context(nc.semaphore("s_tcp"))
        s_tTcp = stack.enter_context(nc.semaphore("s_tTcp"))
        s_o5 = stack.enter_context(nc.semaphore("s_o5"))
        s_rep = stack.enter_context(nc.semaphore("s_rep"))
        s_out = stack.enter_context(nc.semaphore("s_out"))

        with nc.Block(no_gpsimd_drain=True) as block:

            @block.sync
            def _(sync):
                sync.dma_start(
                    out=wv_sb[:, 0, :], in_=wvc[:, 0, :]
                ).then_inc(s_wv[0], 16)
                sync.dma_start(out=ctx_sb[:], in_=ctxc[:]).then_inc(s_ctx, 16)
                for k in range(1, KC):
                    sync.dma_start(
                        out=wv_sb[:, k, :], in_=wvc[:, k, :]
                    ).then_inc(s_wv[k], 16)
                sync.wait_ge(s_rep, 1)
                # pos = r*256 + p*2 + d: each partition contributes 2048-byte
                # contiguous chunks (two consecutive 256-float rows).
                out_view = outd.rearrange("(r p d) n -> p r (d n)", p=P, d=2)
                src_view = (
                    rep_sb.rearrange("p d n -> p (d n)")[:, None, :]
                    .broadcast_to((P, NPOS // (2 * P), 2 * ROW))
                )
                sync.dma_start(out=out_view, in_=src_view).then_inc(s_out, 16)
                sync.wait_ge(s_out, 16)

            @block.scalar
            def _(scalar):
                scalar.dma_start(out=wo_sb[:], in_=woc[:]).then_inc(s_const, 16)
                scalar.dma_start(out=sel_sb[:], in_=selc[:]).then_inc(s_const, 16)
                scalar.dma_start(out=id_sb[:], in_=idc[:]).then_inc(s_const, 16)
                scalar.dma_start(
                    out=o5_sb[B:B + 1, :], in_=boc[:]
                ).then_inc(s_const, 16)

            @block.tensor
            def _(tensor):
                # HAM warmup: ungated dummy matmuls (garbage SBUF data,
                # scratch PSUM) keep the PE busy from boot until the first
                # Wv chunk lands, ramping the PE clock from 1.2 to 2.4 GHz.
                for _w in range(2):
                    nc.tensor.matmul(
                        pwarm[:],
                        ctx_sb[:, 0, :],
                        wv_sb[:, KC - 1, :],
                        start=True,
                        stop=True,
                    )
                tensor.wait_ge(s_ctx, 16)
                for k in range(KC):
                    tensor.wait_ge(s_wv[k], 16)
                    ins = nc.tensor.matmul(
                        pt[:],
                        ctx_sb[:, k, :],
                        wv_sb[:, k, :],
                        start=(k == 0),
                        stop=(k == KC - 1),
                    )
                ins.then_inc(s_mm, 1)

                tensor.wait_ge(s_tcp, 1)
                tensor.wait_ge(s_const, 64)
                for m in range(KD):
                    ins = nc.tensor.transpose(
                        ptT[:, m, :], t_sb[:, m * P:(m + 1) * P], id_sb[:]
                    )
                ins.then_inc(s_mm, 1)

                tensor.wait_ge(s_tTcp, 1)
                for m in range(KD):
                    ins = nc.tensor.matmul(
                        po[:],
                        tT_sb[:, m, :],
                        wo_sb[:, m, :],
                        start=(m == 0),
                        stop=(m == KD - 1),
                    )
                ins.then_inc(s_mm, 1)

                tensor.wait_ge(s_o5, 1)
                for b in range(B):
                    ins = nc.tensor.matmul(
                        prep[:, b, :],
                        sel_sb[:, b, :],
                        o5_sb[:, :],
                        start=True,
                        stop=True,
                    )
                ins.then_inc(s_mm, 1)

            @block.vector
            def _(vector):
                vector.wait_ge(s_mm, 1)
                nc.vector.tensor_copy(t_sb[:], pt[:]).then_inc(s_tcp, 1)
                vector.wait_ge(s_mm, 2)
                nc.vector.tensor_copy(tT_sb[:], ptT[:]).then_inc(s_tTcp, 1)
                vector.wait_ge(s_mm, 3)
                nc.vector.tensor_copy(o5_sb[:B, :], po[:]).then_inc(s_o5, 1)
                vector.wait_ge(s_mm, 4)
                flat = prep[:].rearrange("p b c -> p (b c)")
                nc.vector.tensor_copy(rep_sb[:, 0, :], flat)
                nc.vector.tensor_copy(rep_sb[:, 1, :], flat).then_inc(s_rep, 1)

    nc.compile()
    return nc


def _get_nc():
    if "nc" not in _CACHE:
        _CACHE["nc"] = _build_nc()
    return _CACHE["nc"]


def _prepare_in_maps(context, Wv, Wo, bo):
    context = np.ascontiguousarray(context, dtype=np.float32)
    Wv = np.ascontiguousarray(Wv, dtype=np.float32)
    Wo = np.ascontiguousarray(Wo, dtype=np.float32)
    bo = np.ascontiguousarray(bo, dtype=np.float32)

    ctxc = np.ascontiguousarray(context.T.reshape(KC, P, B).transpose(1, 0, 2))
    wvc = np.ascontiguousarray(Wv.reshape(KC, P, DIM).transpose(1, 0, 2))
    wo_chunk = Wo.reshape(KD, P, DIM).transpose(1, 0, 2)

    selc = np.zeros((B + 1, B, P), dtype=np.float32)
    for b in range(B):
        selc[b, b, :] = 1.0
        selc[B, b, :] = 1.0
    idc = np.eye(B, dtype=np.float32)

    in_maps = []
    for i in range(NCORES):
        woc = np.ascontiguousarray(wo_chunk[:, :, i * CPC:(i + 1) * CPC])
        boc = np.ascontiguousarray(bo[i * CPC:(i + 1) * CPC]).reshape(1, CPC)
        in_maps.append(
            {
                "ctxc": ctxc,
                "wvc": wvc,
                "woc": woc,
                "selc": selc,
                "idc": idc,
                "boc": boc,
            }
        )
    return in_maps


def _unshard(results):
    shards = np.stack([r["outd"] for r in results], axis=0)
    shards = shards.reshape(NCORES, NPOS, B, CPC)
    out = shards.transpose(2, 0, 3, 1).reshape(B, DIM, H, W)
    return np.ascontiguousarray(out)


def kernel(x, context, Wq, Wk, Wv, Wo, bo):
    del x, Wq, Wk
    nc = _get_nc()
    in_maps = _prepare_in_maps(context, Wv, Wo, bo)
    results = run_bass_kernel_spmd(nc, in_maps, list(range(NCORES))).results
    return _unshard(results)
